# revision 6
# baseline (speedup 1.0000x reference)
"""AdaptiveEdgeSmoothing Trainium2 kernel.

Reference semantics (per sample, 1024x1024 f32 image):
    edges     = |conv3x3(mask, LAPLACIAN)|          (SAME zero pad)
    edge_mask = edges > 0.5*edge_sensitivity
    sm        = mask*(1-bf) + box5(mask)/25*bf,  bf = blur_strength/3
    result    = where(edge_mask, sm, mask)
    out       = (result > final_threshold).astype(f32)

Strategy: B=16 samples sharded 2-per-core across 8 NeuronCores (pure data
parallel).  Per core, each image is processed in 9 row-tiles (rows on
partitions, cols on the free axis).  All convolution arithmetic runs on the
TensorEngine as banded fp32r matmuls over column-shifted rhs views of a
zero-margined SBUF tile:
    PSUM1 = 9x - box3(x)            (3 accumulating passes; the Laplacian)
    PSUM2 = (bf/25)*box5(x)+(1-bf)x (5 passes; the smoothed value)
Vertical band weights (including top/bottom SAME-padding clipping and the
per-sample bf scaling) are precomputed in numpy and DMA'd in.  The halo rows
(2 above the tile) are parked at spare partitions so that output rows start
at partition 0 on every operand.  Elementwise tail: ACT computes
Relu(|lap| - thr) as an edge mask (nonzero = edge), DVE copy_predicated
overwrites x with sm where masked, then one is_gt against final_threshold.
"""

import sys

if '/opt/trn_rl_repo' not in sys.path:
    sys.path.insert(0, '/opt/trn_rl_repo')

import numpy as np

import concourse.bacc as bacc
import concourse.mybir as mybir
from concourse.tile import TileContext
from concourse.bass_utils import run_bass_kernel_spmd

H = W = 1024
N_CORES = 8
IMGS_PER_CORE = 2
F32 = mybir.dt.float32
F32R = mybir.dt.float32r

# tile geometry: (out_row_start, n_out, K_data, halo_partition_base, has_top)
# partitions [0, K_data) hold rows [s, s+K_data); partitions
# [halo_base, halo_base+2) hold rows [s-2, s) when has_top.
TILES = []
for t in range(8):
    TILES.append((124 * t, 124, 126, 126, t > 0))
TILES.append((992, 32, 32, 32, True))
TILE_VARIANTS = [0] + [1] * 7 + [2]  # first / mid / last weight variant


def _band_templates():
    """Per variant: (V3, V5, I) as [128,128] f32, plus (K_total, nout)."""
    out = []
    for var in range(3):
        s, nout, kd, hb, has_top = TILES[0 if var == 0 else (1 if var == 1 else 8)]
        v3 = np.zeros((128, 128), np.float32)
        v5 = np.zeros((128, 128), np.float32)
        ident = np.zeros((128, 128), np.float32)
        for k in range(kd):
            for p in range(nout):
                d = k - p
                if abs(d) <= 1:
                    v3[k, p] = 1.0
                if abs(d) <= 2:
                    v5[k, p] = 1.0
                if d == 0:
                    ident[k, p] = 1.0
        if var != 0:  # top halo rows: partition hb+j holds row s-2+j
            for j in range(2):
                for p in range(nout):
                    d = (j - 2) - p  # row offset relative to output row p
                    if abs(d) <= 1:
                        v3[hb + j, p] = 1.0
                    if abs(d) <= 2:
                        v5[hb + j, p] = 1.0
        k_tot = 128 if var != 2 else 34
        out.append((v3, v5, ident, k_tot, nout))
    return out


_TEMPLATES = _band_templates()

_compiled = None
last_results = None


def _build():
    nc = bacc.Bacc("TRN2", target_bir_lowering=False, debug=False,
                   num_devices=N_CORES)
    x = nc.dram_tensor("x", [IMGS_PER_CORE, H, W], F32R,
                       kind="ExternalInput").ap()
    w3 = nc.dram_tensor("w3", [3, 2, 128, 128], F32R,
                        kind="ExternalInput").ap()
    w5 = nc.dram_tensor("w5", [IMGS_PER_CORE, 3, 2, 128, 128], F32R,
                        kind="ExternalInput").ap()
    negthr = nc.dram_tensor("negthr", [IMGS_PER_CORE, 128, 1], F32,
                            kind="ExternalInput").ap()
    ft = nc.dram_tensor("ft", [IMGS_PER_CORE, 128, 1], F32,
                        kind="ExternalInput").ap()
    y = nc.dram_tensor("out", [IMGS_PER_CORE, H, W], F32,
                       kind="ExternalOutput").ap()

    with TileContext(nc) as tc:
        with (
            tc.tile_pool(name="wpool", bufs=1) as wpool,
            tc.tile_pool(name="spool", bufs=1) as spool,
            tc.tile_pool(name="xpool", bufs=3) as xpool,
            tc.tile_pool(name="p1pool", bufs=2, space="PSUM") as p1pool,
            tc.tile_pool(name="p2pool", bufs=2, space="PSUM") as p2pool,
            tc.tile_pool(name="apool", bufs=2) as apool,
            tc.tile_pool(name="empool", bufs=2) as empool,
            tc.tile_pool(name="vpool", bufs=2) as vpool,
            tc.tile_pool(name="opool", bufs=3) as opool,
        ):
            # --- one-time loads: weights + per-image scalars -------------
            w3_t = []  # [variant] -> (side, center)
            for v in range(3):
                side = wpool.tile([128, 128], F32R, tag=f"w3s{v}")
                cent = wpool.tile([128, 128], F32R, tag=f"w3c{v}")
                nc.sync.dma_start(out=side[:], in_=w3[v, 0])
                nc.sync.dma_start(out=cent[:], in_=w3[v, 1])
                w3_t.append((side, cent))
            w5_t = []  # [img][variant] -> (side, center)
            for img in range(IMGS_PER_CORE):
                per = []
                for v in range(3):
                    side = wpool.tile([128, 128], F32R, tag=f"w5s{img}{v}")
                    cent = wpool.tile([128, 128], F32R, tag=f"w5c{img}{v}")
                    nc.sync.dma_start(out=side[:], in_=w5[img, v, 0])
                    nc.sync.dma_start(out=cent[:], in_=w5[img, v, 1])
                    per.append((side, cent))
                w5_t.append(per)
            sc_t = []  # [img] -> (negthr, ft)
            for img in range(IMGS_PER_CORE):
                nt = spool.tile([128, 1], F32, tag=f"nt{img}")
                f = spool.tile([128, 1], F32, tag=f"ft{img}")
                nc.sync.dma_start(out=nt[:], in_=negthr[img])
                nc.sync.dma_start(out=f[:], in_=ft[img])
                sc_t.append((nt, f))

            # --- main loop ----------------------------------------------
            for img in range(IMGS_PER_CORE):
                nt_ap, ft_ap = sc_t[img]
                for t, (s, nout, kd, hb, has_top) in enumerate(TILES):
                    var = TILE_VARIANTS[t]
                    k_tot = _TEMPLATES[var][3]
                    xb = xpool.tile([128, 1028], F32R, tag="xb")
                    # zero margins for the horizontal SAME-pad shifts
                    nc.gpsimd.memset(xb[:, 0:2].bitcast(F32), 0)
                    nc.gpsimd.memset(xb[:, 1026:1028].bitcast(F32), 0)
                    # main rows -> partitions [0, kd)
                    nc.sync.dma_start(out=xb[0:kd, 2:1026],
                                      in_=x[img, s:s + kd, :])
                    # top halo rows s-2,s-1 -> partitions [hb, hb+2)
                    # (t=0 loads rows 0,1 there; weights are zero for them)
                    hs = s - 2 if has_top else 0
                    nc.sync.dma_start(out=xb[hb:hb + 2, 2:1026],
                                      in_=x[img, hs:hs + 2, :])

                    p1 = p1pool.tile([128, 1024], F32, tag="p1")
                    p2 = p2pool.tile([128, 1024], F32, tag="p2")
                    w3s, w3c = w3_t[var]
                    w5s, w5c = w5_t[img][var]
                    for c in (0, 512):
                        for i, sh in enumerate((-1, 1, 0)):
                            wt = w3c if sh == 0 else w3s
                            nc.tensor.matmul(
                                p1[0:nout, c:c + 512],
                                wt[0:k_tot, 0:nout],
                                xb[0:k_tot, 2 + sh + c:2 + sh + c + 512],
                                start=(i == 0), stop=(i == 2))
                        for i, sh in enumerate((-2, -1, 1, 2, 0)):
                            wt = w5c if sh == 0 else w5s
                            nc.tensor.matmul(
                                p2[0:nout, c:c + 512],
                                wt[0:k_tot, 0:nout],
                                xb[0:k_tot, 2 + sh + c:2 + sh + c + 512],
                                start=(i == 0), stop=(i == 4))

                    # edge mask: nonzero where |lap| > thr
                    a_t = apool.tile([128, 1024], F32, tag="a")
                    em_t = empool.tile([128, 1024], F32, tag="em")
                    nc.scalar.activation(a_t[0:nout, :], p1[0:nout, :],
                                         mybir.ActivationFunctionType.Abs)
                    nc.scalar.activation(em_t[0:nout, :], a_t[0:nout, :],
                                         mybir.ActivationFunctionType.Relu,
                                         bias=nt_ap[0:nout, :])
                    # v = x; v <- sm where edge; out = (v > ft)
                    v_t = vpool.tile([128, 1024], F32, tag="v")
                    nc.vector.tensor_copy(v_t[0:nout, :],
                                          xb[0:nout, 2:1026].bitcast(F32))
                    nc.vector.copy_predicated(v_t[0:nout, :],
                                              em_t[0:nout, :]
                                              .bitcast(mybir.dt.int32),
                                              p2[0:nout, 0:1024])
                    o_t = opool.tile([128, 1024], F32, tag="o")
                    nc.vector.tensor_scalar(o_t[0:nout, :],
                                            v_t[0:nout, :],
                                            ft_ap[0:nout, :], None,
                                            mybir.AluOpType.is_gt)
                    nc.sync.dma_start(out=y[img, s:s + nout, :],
                                      in_=o_t[0:nout, :])
    nc.compile()
    return nc


def _in_maps(mask, blur_strength, edge_sensitivity, final_threshold):
    mask = np.ascontiguousarray(mask.reshape(16, H, W), np.float32)
    bs = np.asarray(blur_strength, np.float32).reshape(16)
    es = np.asarray(edge_sensitivity, np.float32).reshape(16)
    fts = np.asarray(final_threshold, np.float32).reshape(16)

    w3 = np.zeros((3, 2, 128, 128), np.float32)
    for v, (v3, v5, ident, k_tot, nout) in enumerate(_TEMPLATES):
        w3[v, 0] = -v3
        w3[v, 1] = 9.0 * ident - v3

    maps = []
    for c in range(N_CORES):
        sel = slice(2 * c, 2 * c + 2)
        w5 = np.zeros((IMGS_PER_CORE, 3, 2, 128, 128), np.float32)
        for i in range(IMGS_PER_CORE):
            bf = bs[2 * c + i] / 3.0
            for v, (v3, v5t, ident, k_tot, nout) in enumerate(_TEMPLATES):
                w5[i, v, 0] = (bf / 25.0) * v5t
                w5[i, v, 1] = (bf / 25.0) * v5t + (1.0 - bf) * ident
        negthr = np.zeros((IMGS_PER_CORE, 128, 1), np.float32)
        ftm = np.zeros((IMGS_PER_CORE, 128, 1), np.float32)
        for i in range(IMGS_PER_CORE):
            negthr[i, :, 0] = -(0.5 * es[2 * c + i])
            ftm[i, :, 0] = fts[2 * c + i]
        maps.append({
            "x": np.ascontiguousarray(mask[sel]),
            "w3": w3,
            "w5": w5,
            "negthr": negthr,
            "ft": ftm,
        })
    return maps


def kernel(mask, blur_strength, edge_sensitivity, final_threshold):
    global _compiled, last_results
    if _compiled is None:
        _compiled = _build()
    maps = _in_maps(mask, blur_strength, edge_sensitivity, final_threshold)
    res = run_bass_kernel_spmd(_compiled, maps, core_ids=list(range(N_CORES)))
    last_results = res
    out = np.empty((16, 1, H, W), np.float32)
    for c in range(N_CORES):
        out[2 * c:2 * c + 2, 0] = res.results[c]["out"]
    return out
